# revision 1
# baseline (speedup 1.0000x reference)
"""TRN2 Bass kernel for the quantized 4-layer MLP (dense_mlp, 8 cores).

Strategy:
  - Data-parallel over batch: each of the 8 cores gets 1024 of 8192 rows.
  - Activations kept transposed [feature, batch] on-chip the whole way;
    quantized activation LEVELS (ints 0..15) stored as fp8e4 (exact).
  - Layer 1: x split into fp16 hi+lo parts (2 fp16 matmuls; 22+ mantissa
    bits total => fp32-level accuracy), weights quantized to ints -3..3
    (exact in fp16).
  - Layers 2-4: fp8e4 DoubleRow matmuls over integer levels - bit-exact,
    2x tensor-engine throughput.
  - Weight quantization (round(W/s)) done on device: ACT affine with +C
    round trick, DVE -C with dtype cast.
  - BN + QuantReLU epilogue fused: ACT per-feature affine, DVE round(+C,-C),
    DVE clip(min 15, max 0) with fp8 output cast.
  - Weight staging pools for layers 2-4 are shared and long-lived so the
    next layer's weight DMA + quant prefetches during the current layer.
"""

import numpy as np
import ml_dtypes  # noqa: F401

B, D_IN, H, C_OUT = 8192, 2048, 4096, 1000
NCORES = 8
BC = B // NCORES            # 1024 batch rows per core
N4P = 1024                  # padded final output feature dim (1000 -> 1024)
C_ROUND = float(1.5 * 2 ** 23)
EPS = 1e-5

_CACHE = {}


def _build_nc():
    import concourse.bass as bass  # noqa: F401
    from concourse import bacc
    import concourse.mybir as mybir
    import concourse.tile as tile

    dt = mybir.dt
    P = 128
    AF = mybir.ActivationFunctionType
    ALU = mybir.AluOpType

    nc = bacc.Bacc("TRN2", target_bir_lowering=False)

    # ---- DRAM I/O ----
    xt_hi = nc.dram_tensor("xt_hi", [D_IN, BC], dt.float16, kind="ExternalInput")
    xt_lo = nc.dram_tensor("xt_lo", [D_IN, BC], dt.float16, kind="ExternalInput")
    w1t = nc.dram_tensor("w1t", [D_IN, H], dt.float32, kind="ExternalInput")
    w2t = nc.dram_tensor("w2t", [H, H], dt.float32, kind="ExternalInput")
    w3t = nc.dram_tensor("w3t", [H, H], dt.float32, kind="ExternalInput")
    w4t = nc.dram_tensor("w4t", [H, N4P], dt.float32, kind="ExternalInput")
    ab1 = nc.dram_tensor("ab1", [H, 2], dt.float32, kind="ExternalInput")
    ab2 = nc.dram_tensor("ab2", [H, 2], dt.float32, kind="ExternalInput")
    ab3 = nc.dram_tensor("ab3", [H, 2], dt.float32, kind="ExternalInput")
    ab4 = nc.dram_tensor("ab4", [N4P, 2], dt.float32, kind="ExternalInput")
    inv_sb = nc.dram_tensor("inv_sb", [P, 4], dt.float32, kind="ExternalInput")
    out_t = nc.dram_tensor("out_t", [N4P, BC], dt.float32, kind="ExternalOutput")

    with tile.TileContext(nc) as tc:
        ppool_cm = tc.tile_pool(name="psum", bufs=8, space="PSUM")
        ppool = ppool_cm.__enter__()
        const_cm = tc.tile_pool(name="const", bufs=1)
        cpool = const_cm.__enter__()

        invs = cpool.tile([P, 4], dt.float32, name="invs")
        nc.gpsimd.dma_start(invs[:], inv_sb[:])
        cbias = cpool.tile([P, 1], dt.float32, name="cbias")
        nc.vector.memset(cbias[:], C_ROUND)

        # A1 (layer-1 output levels) - lives through L2
        apool12_cm = tc.tile_pool(name="acts12", bufs=1)
        apool12 = apool12_cm.__enter__()
        A1 = apool12.tile([P, H // P, BC], dt.float8e4, name="A1")

        # long-lived weight staging pools shared by layers 2-4 (enables
        # cross-layer prefetch of weight DMA + quantization)
        wf234_cm = tc.tile_pool(name="wf234", bufs=6)
        wf234 = wf234_cm.__enter__()
        qt234_cm = tc.tile_pool(name="qt234", bufs=4)
        qt234 = qt234_cm.__enter__()
        tmp_cm = tc.tile_pool(name="tmpp", bufs=3)
        tmp_pool = tmp_cm.__enter__()
        abt_cm = tc.tile_pool(name="abtp", bufs=1)
        abt_pool = abt_cm.__enter__()

        def layer(wt, ab, K, N, mode, a_in, a_out, inv_idx, wf_pool, qt_pool,
                  wq_dt, out_stage_pool=None):
            KT = K // P
            NT = N // P
            abt = abt_pool.tile([P, NT, 2], dt.float32, name=f"abt{inv_idx}",
                                tag=f"abt{inv_idx}")
            nc.gpsimd.dma_start(abt[:], ab[:].rearrange("(nt p) two -> p nt two", p=P))

            for nt in range(NT):
                n0 = nt * P
                qt = qt_pool.tile([P, KT, P], wq_dt,
                                  name=f"qt{inv_idx}", tag="qt")
                wsrc = wt[:, n0:n0 + P].rearrange("(kt p) n -> p kt n", p=P)
                step = KT // 2
                for h in range(2):
                    sl = slice(h * step, (h + 1) * step)
                    wf = wf_pool.tile([P, step, P], dt.float32,
                                      name=f"wf{inv_idx}", tag="wf")
                    nc.sync.dma_start(wf[:], wsrc[:, sl, :])
                    # quantize: ACT computes round(W*inv_s) + C in place (fp32)
                    nc.scalar.activation(
                        wf[:], wf[:], AF.Identity,
                        bias=cbias[:], scale=invs[:, inv_idx:inv_idx + 1],
                    )
                    nc.vector.tensor_scalar(qt[:, sl, :], wf[:],
                                            C_ROUND, None, ALU.subtract)

                for b0 in range(0, BC, 512):
                    psum = ppool.tile([P, 512], dt.float32, name="psum", tag="ps")
                    if mode == "hilo":
                        for c0 in range(0, KT, 4):
                            for pi in range(2):
                                for kt in range(c0, c0 + 4):
                                    nc.tensor.matmul(
                                        psum[:], qt[:, kt, :],
                                        a_in[pi][:, kt, b0:b0 + 512],
                                        start=(c0 == 0 and pi == 0 and kt == 0),
                                        stop=(c0 + 4 == KT and pi == 1 and kt == KT - 1))
                    else:
                        for kp in range(KT // 2):
                            nc.tensor.matmul(
                                psum[:], qt[:, 2 * kp:2 * kp + 2, :],
                                a_in[:, 2 * kp:2 * kp + 2, b0:b0 + 512],
                                start=(kp == 0), stop=(kp == KT // 2 - 1),
                                perf_mode=mybir.MatmulPerfMode.DoubleRow)

                    if a_out is not None:
                        tmp = tmp_pool.tile([P, 512], dt.float32, name="tmp", tag="tmp")
                        nc.scalar.activation(
                            tmp[:], psum[:], AF.Identity,
                            bias=abt[:, nt, 1:2], scale=abt[:, nt, 0:1])
                        nc.vector.tensor_scalar(tmp[:], tmp[:], C_ROUND, C_ROUND,
                                                ALU.add, ALU.subtract)
                        nc.vector.tensor_scalar(a_out[:, nt, b0:b0 + 512], tmp[:],
                                                15.0, 0.0, ALU.min, ALU.max)
                    else:
                        ost = out_stage_pool.tile([P, 512], dt.float32,
                                                  name="ost", tag="ost")
                        nc.scalar.activation(
                            ost[:], psum[:], AF.Identity,
                            bias=abt[:, nt, 1:2], scale=abt[:, nt, 0:1])
                        nc.gpsimd.dma_start(out_t[n0:n0 + P, b0:b0 + 512], ost[:])

        # ---- layer 1 (fp16 hi/lo) ----
        xt_pool_cm = tc.tile_pool(name="xtp", bufs=1)
        xt_pool = xt_pool_cm.__enter__()
        xh = xt_pool.tile([P, D_IN // P, BC], dt.float16, name="xh")
        xl = xt_pool.tile([P, D_IN // P, BC], dt.float16, name="xl")
        xh_src = xt_hi[:].rearrange("(kt p) b -> p kt b", p=P)
        xl_src = xt_lo[:].rearrange("(kt p) b -> p kt b", p=P)
        KTX = D_IN // P
        for c0 in range(0, KTX, 4):
            nc.sync.dma_start(xh[:, c0:c0 + 4, :], xh_src[:, c0:c0 + 4, :])
            nc.gpsimd.dma_start(xl[:, c0:c0 + 4, :], xl_src[:, c0:c0 + 4, :])

        w1f_cm = tc.tile_pool(name="w1f", bufs=4)
        w1f = w1f_cm.__enter__()
        q1_cm = tc.tile_pool(name="q1", bufs=2)
        q1 = q1_cm.__enter__()

        layer(w1t, ab1, D_IN, H, "hilo", (xh, xl), A1, 0, w1f, q1, dt.float16)

        q1_cm.__exit__(None, None, None)
        w1f_cm.__exit__(None, None, None)
        xt_pool_cm.__exit__(None, None, None)

        # ---- layers 2-4 (fp8 DoubleRow) ----
        apool23_cm = tc.tile_pool(name="acts23", bufs=1)
        apool23 = apool23_cm.__enter__()
        A2 = apool23.tile([P, H // P, BC], dt.float8e4, name="A2")
        layer(w2t, ab2, H, H, "dr", A1, A2, 1, wf234, qt234, dt.float8e4)

        A3 = apool23.tile([P, H // P, BC], dt.float8e4, name="A3")
        layer(w3t, ab3, H, H, "dr", A2, A3, 2, wf234, qt234, dt.float8e4)

        ost_cm = tc.tile_pool(name="ostp", bufs=3)
        ost_pool = ost_cm.__enter__()
        layer(w4t, ab4, H, N4P, "dr", A3, None, 3, wf234, qt234, dt.float8e4,
              out_stage_pool=ost_pool)

        ost_cm.__exit__(None, None, None)
        apool23_cm.__exit__(None, None, None)
        abt_cm.__exit__(None, None, None)
        tmp_cm.__exit__(None, None, None)
        qt234_cm.__exit__(None, None, None)
        wf234_cm.__exit__(None, None, None)
        apool12_cm.__exit__(None, None, None)
        const_cm.__exit__(None, None, None)
        ppool_cm.__exit__(None, None, None)

    nc.compile()
    return nc


def _host_prep(inputs):
    f32 = np.float32

    def wscale(W):
        # mimic reference: s = max(|W|) / 3.0 in fp32
        return f32(np.max(np.abs(W))) / f32(3.0)

    s_w = [wscale(inputs[k]) for k in ("W1", "W2", "W3", "W4")]
    s_a = [f32(inputs[k][0]) for k in ("s1", "s2", "s3")]

    # per-feature affine folds (fp64 then cast once to fp32)
    def fold(l, s_prev):
        g = inputs[f"g{l}"].astype(np.float64)
        be = inputs[f"be{l}"].astype(np.float64)
        m = inputs[f"m{l}"].astype(np.float64)
        v = inputs[f"v{l}"].astype(np.float64)
        b = inputs[f"b{l}"].astype(np.float64)
        inv = 1.0 / np.sqrt(v + EPS)
        sl = float(s_a[l - 1])
        alpha = (float(s_prev) * float(s_w[l - 1]) * g * inv) / sl
        beta = ((b - m) * inv * g + be) / sl
        return alpha.astype(f32), beta.astype(f32)

    a1, b1 = fold(1, 1.0)
    a2, b2 = fold(2, s_a[0])
    a3, b3 = fold(3, s_a[1])
    a4 = np.full(N4P, float(s_a[2]) * float(s_w[3]), dtype=f32)
    b4 = np.zeros(N4P, dtype=f32)
    b4[:C_OUT] = inputs["b4"]

    def abpack(a, b):
        return np.ascontiguousarray(np.stack([a, b], axis=1))

    w1t = np.ascontiguousarray(inputs["W1"].T)
    w2t = np.ascontiguousarray(inputs["W2"].T)
    w3t = np.ascontiguousarray(inputs["W3"].T)
    w4t = np.zeros((H, N4P), dtype=f32)
    w4t[:, :C_OUT] = inputs["W4"].T

    inv_sb = np.broadcast_to(
        np.array([1.0 / s for s in s_w], dtype=f32)[None, :], (128, 4)
    ).copy()

    xt = inputs["x"].T  # [D_IN, B] view
    shared = dict(
        w1t=w1t, w2t=w2t, w3t=w3t, w4t=w4t,
        ab1=abpack(a1, b1), ab2=abpack(a2, b2), ab3=abpack(a3, b3),
        ab4=abpack(a4, b4),
        inv_sb=inv_sb,
    )
    in_maps = []
    for c in range(NCORES):
        xs = np.ascontiguousarray(xt[:, c * BC:(c + 1) * BC], dtype=f32)
        xhi = xs.astype(np.float16)
        xlo = (xs - xhi.astype(f32)).astype(np.float16)
        m = dict(shared)
        m["xt_hi"] = xhi
        m["xt_lo"] = xlo
        in_maps.append(m)
    return in_maps


def kernel(**inputs):
    from concourse.bass_utils import run_bass_kernel_spmd

    inputs = {k: np.asarray(v) for k, v in inputs.items()}
    if "nc" not in _CACHE:
        _CACHE["nc"] = _build_nc()
    nc = _CACHE["nc"]

    in_maps = _host_prep(inputs)
    res = run_bass_kernel_spmd(nc, in_maps, core_ids=list(range(NCORES)))

    out = np.empty((B, C_OUT), dtype=np.float32)
    for c in range(NCORES):
        out[c * BC:(c + 1) * BC, :] = res.results[c]["out_t"][:C_OUT, :].T
    return out



# revision 3
# speedup vs baseline: 1.1807x; 1.1807x over previous
"""TRN2 Bass kernel for the quantized 4-layer MLP (dense_mlp, 8 cores).

Strategy (v2):
  - Data-parallel over batch: each of the 8 cores gets 1024 of 8192 rows.
  - All weights quantized to integer LEVELS on host (bit-exact replica of
    the reference wquant: round(W/s) with RTNE), shipped as fp16 (layer-1
    hi) / fp8e4 (everything else). No on-device weight quantization.
  - Layer 1 x split: hi = fp16(x) [16 matmuls/tile], lo = fp8(r*2^9) with
    stationary levels*2^-9 (exact in fp8e4 subnormals) via DoubleRow
    [8 matmuls/tile]. hi/lo accumulate in separate PSUM banks (mixing
    perf modes in one accumulation group is broken on HW) and are summed
    by a DVE scalar_tensor_tensor in the epilogue. Combined x precision
    ~2^-15; simulated end-to-end rel err 1.04e-2 (gate 2e-2).
  - Layers 2-4: fp8e4 DoubleRow matmuls over integer levels - bit-exact,
    2x tensor-engine throughput.
  - BN + QuantReLU epilogue fused: ACT per-feature affine, DVE round
    (+C/-C trick), DVE clip(15,0) with fp8 output cast.
  - Total matmuls/core: 1536 (L1) + 1024 (L2) + 1024 (L3) + 256 (L4)
    = 3840 @ ~216ns issue rate -> ~830us floor.
"""

import numpy as np
import ml_dtypes

B, D_IN, H, C_OUT = 8192, 2048, 4096, 1000
NCORES = 8
BC = B // NCORES            # 1024 batch rows per core
N4P = 1024                  # padded final output feature dim (1000 -> 1024)
C_ROUND = float(1.5 * 2 ** 23)
EPS = 1e-5
LO_SC = 512.0               # 2^9 residual scale for the fp8 lo pass

_CACHE = {}


def _build_nc():
    import concourse.bass as bass  # noqa: F401
    from concourse import bacc
    import concourse.mybir as mybir
    import concourse.tile as tile

    dt = mybir.dt
    P = 128
    AF = mybir.ActivationFunctionType
    ALU = mybir.AluOpType
    DR = mybir.MatmulPerfMode.DoubleRow

    nc = bacc.Bacc("TRN2", target_bir_lowering=False)

    # ---- DRAM I/O ----
    xh_d = nc.dram_tensor("xh", [D_IN, BC], dt.float16, kind="ExternalInput")
    xl_d = nc.dram_tensor("xl", [D_IN, BC], dt.float8e4, kind="ExternalInput")
    w1h_d = nc.dram_tensor("w1h", [D_IN, H], dt.float16, kind="ExternalInput")
    w1l_d = nc.dram_tensor("w1l", [D_IN, H], dt.float8e4, kind="ExternalInput")
    w2_d = nc.dram_tensor("w2", [H, H], dt.float8e4, kind="ExternalInput")
    w3_d = nc.dram_tensor("w3", [H, H], dt.float8e4, kind="ExternalInput")
    w4_d = nc.dram_tensor("w4", [H, N4P], dt.float8e4, kind="ExternalInput")
    ab1 = nc.dram_tensor("ab1", [H, 2], dt.float32, kind="ExternalInput")
    ab2 = nc.dram_tensor("ab2", [H, 2], dt.float32, kind="ExternalInput")
    ab3 = nc.dram_tensor("ab3", [H, 2], dt.float32, kind="ExternalInput")
    ab4 = nc.dram_tensor("ab4", [N4P, 2], dt.float32, kind="ExternalInput")
    out_t = nc.dram_tensor("out_t", [N4P, BC], dt.float32, kind="ExternalOutput")

    with tile.TileContext(nc) as tc:
        ppool_cm = tc.tile_pool(name="psum", bufs=4, space="PSUM")
        ppool = ppool_cm.__enter__()

        abt_cm = tc.tile_pool(name="abtp", bufs=1)
        abt_pool = abt_cm.__enter__()
        tmp_cm = tc.tile_pool(name="tmpp", bufs=4)
        tmp_pool = tmp_cm.__enter__()

        # activations (levels) live across layer boundaries
        apool12_cm = tc.tile_pool(name="acts12", bufs=1)
        apool12 = apool12_cm.__enter__()
        A1 = apool12.tile([P, H // P, BC], dt.float8e4, name="A1")

        def epilogue(psum, abt, nt, b0, a_out, lo_psum=None):
            tmp = tmp_pool.tile([P, 512], dt.float32, name="tmp", tag="tmp")
            if a_out is not None:
                nc.scalar.activation(
                    tmp[:], psum[:], AF.Identity,
                    bias=abt[:, nt, 1:2], scale=abt[:, nt, 0:1])
                if lo_psum is not None:
                    # tmp = lo_psum * alpha + tmp  (one PSUM input max)
                    nc.vector.scalar_tensor_tensor(
                        tmp[:], lo_psum[:], abt[:, nt, 0:1], tmp[:],
                        ALU.mult, ALU.add)
                nc.vector.tensor_scalar(tmp[:], tmp[:], C_ROUND, C_ROUND,
                                        ALU.add, ALU.subtract)
                nc.vector.tensor_scalar(a_out[:, nt, b0:b0 + 512], tmp[:],
                                        15.0, 0.0, ALU.min, ALU.max)
            else:
                ost = tmp_pool.tile([P, 512], dt.float32, name="ost", tag="ost")
                nc.scalar.activation(
                    ost[:], psum[:], AF.Identity,
                    bias=abt[:, nt, 1:2], scale=abt[:, nt, 0:1])
                n0 = nt * P
                nc.gpsimd.dma_start(out_t[n0:n0 + P, b0:b0 + 512], ost[:])

        # ---- layer 1: fp16 hi + fp8 DR lo, separate psums ----
        xt_pool_cm = tc.tile_pool(name="xtp", bufs=1)
        xt_pool = xt_pool_cm.__enter__()
        KTX = D_IN // P  # 16
        xh_t = xt_pool.tile([P, KTX, BC], dt.float16, name="xh_t")
        xl_t = xt_pool.tile([P, KTX, BC], dt.float8e4, name="xl_t")
        xh_src = xh_d[:].rearrange("(kt p) b -> p kt b", p=P)
        xl_src = xl_d[:].rearrange("(kt p) b -> p kt b", p=P)
        for c0 in range(0, KTX, 4):
            nc.sync.dma_start(xh_t[:, c0:c0 + 4, :], xh_src[:, c0:c0 + 4, :])
            nc.gpsimd.dma_start(xl_t[:, c0:c0 + 4, :], xl_src[:, c0:c0 + 4, :])

        abt1 = abt_pool.tile([P, H // P, 2], dt.float32, name="abt1")
        nc.gpsimd.dma_start(abt1[:], ab1[:].rearrange("(nt p) two -> p nt two", p=P))

        w1_cm = tc.tile_pool(name="w1p", bufs=3)
        w1_pool = w1_cm.__enter__()
        NT1 = H // P  # 32
        for nt in range(NT1):
            n0 = nt * P
            w1h_t = w1_pool.tile([P, KTX, P], dt.float16, name="w1h_t", tag="wh")
            w1l_t = w1_pool.tile([P, KTX, P], dt.float8e4, name="w1l_t", tag="wl")
            nc.sync.dma_start(
                w1h_t[:], w1h_d[:, n0:n0 + P].rearrange("(kt p) n -> p kt n", p=P))
            nc.sync.dma_start(
                w1l_t[:], w1l_d[:, n0:n0 + P].rearrange("(kt p) n -> p kt n", p=P))
            for b0 in (0, 512):
                ps_h = ppool.tile([P, 512], dt.float32, name="ps_h", tag="ph")
                ps_l = ppool.tile([P, 512], dt.float32, name="ps_l", tag="pl")
                for kt in range(KTX):
                    nc.tensor.matmul(
                        ps_h[:], w1h_t[:, kt, :], xh_t[:, kt, b0:b0 + 512],
                        start=(kt == 0), stop=(kt == KTX - 1))
                for kp in range(KTX // 2):
                    nc.tensor.matmul(
                        ps_l[:], w1l_t[:, 2 * kp:2 * kp + 2, :],
                        xl_t[:, 2 * kp:2 * kp + 2, b0:b0 + 512],
                        start=(kp == 0), stop=(kp == KTX // 2 - 1),
                        perf_mode=DR)
                epilogue(ps_h, abt1, nt, b0, A1, lo_psum=ps_l)

        w1_cm.__exit__(None, None, None)
        xt_pool_cm.__exit__(None, None, None)

        # ---- layers 2-4: fp8 DR with preloaded level weights ----
        apool23_cm = tc.tile_pool(name="acts23", bufs=1)
        apool23 = apool23_cm.__enter__()
        A2 = apool23.tile([P, H // P, BC], dt.float8e4, name="A2")
        A3 = apool23.tile([P, H // P, BC], dt.float8e4, name="A3")

        wf_cm = tc.tile_pool(name="wfp", bufs=3)
        wf_pool = wf_cm.__enter__()

        def layer_dr(wt, ab, N, a_in, a_out, idx):
            KT = H // P  # 32
            NT = N // P
            abt = abt_pool.tile([P, NT, 2], dt.float32, name=f"abt{idx}",
                                tag=f"abt{idx}")
            nc.gpsimd.dma_start(
                abt[:], ab[:].rearrange("(nt p) two -> p nt two", p=P))
            for nt in range(NT):
                n0 = nt * P
                qt = wf_pool.tile([P, KT, P], dt.float8e4, name=f"qt{idx}",
                                  tag="qt")
                nc.sync.dma_start(
                    qt[:], wt[:, n0:n0 + P].rearrange("(kt p) n -> p kt n", p=P))
                for b0 in (0, 512):
                    psum = ppool.tile([P, 512], dt.float32, name="ps", tag="ph")
                    for kp in range(KT // 2):
                        nc.tensor.matmul(
                            psum[:], qt[:, 2 * kp:2 * kp + 2, :],
                            a_in[:, 2 * kp:2 * kp + 2, b0:b0 + 512],
                            start=(kp == 0), stop=(kp == KT // 2 - 1),
                            perf_mode=DR)
                    epilogue(psum, abt, nt, b0, a_out)

        layer_dr(w2_d, ab2, H, A1, A2, 2)
        layer_dr(w3_d, ab3, H, A2, A3, 3)
        layer_dr(w4_d, ab4, N4P, A3, None, 4)

        wf_cm.__exit__(None, None, None)
        apool23_cm.__exit__(None, None, None)
        apool12_cm.__exit__(None, None, None)
        tmp_cm.__exit__(None, None, None)
        abt_cm.__exit__(None, None, None)
        ppool_cm.__exit__(None, None, None)

    nc.compile()
    return nc


def _host_prep(inputs):
    f32 = np.float32
    fp8 = ml_dtypes.float8_e4m3

    def levels(W):
        s = f32(np.max(np.abs(W))) / f32(3.0)
        return np.clip(np.round(W / s), -3.0, 3.0).astype(f32), s

    L1, sw1 = levels(inputs["W1"])
    L2, sw2 = levels(inputs["W2"])
    L3, sw3 = levels(inputs["W3"])
    L4, sw4 = levels(inputs["W4"])
    s_a = [f32(inputs[k][0]) for k in ("s1", "s2", "s3")]

    def fold(l, s_w, s_prev):
        g = inputs[f"g{l}"].astype(np.float64)
        be = inputs[f"be{l}"].astype(np.float64)
        m = inputs[f"m{l}"].astype(np.float64)
        v = inputs[f"v{l}"].astype(np.float64)
        b = inputs[f"b{l}"].astype(np.float64)
        inv = 1.0 / np.sqrt(v + EPS)
        sl = float(s_a[l - 1])
        alpha = (float(s_prev) * float(s_w) * g * inv) / sl
        beta = ((b - m) * inv * g + be) / sl
        return alpha.astype(f32), beta.astype(f32)

    a1, b1 = fold(1, sw1, 1.0)
    a2, b2 = fold(2, sw2, s_a[0])
    a3, b3 = fold(3, sw3, s_a[1])
    a4 = np.full(N4P, float(s_a[2]) * float(sw4), dtype=f32)
    b4 = np.zeros(N4P, dtype=f32)
    b4[:C_OUT] = inputs["b4"]

    def abpack(a, b):
        return np.ascontiguousarray(np.stack([a, b], axis=1))

    w1h = np.ascontiguousarray(L1.T).astype(np.float16)       # [D_IN, H] levels
    w1l = np.ascontiguousarray(L1.T * f32(1.0 / LO_SC)).astype(fp8)
    w2 = np.ascontiguousarray(L2.T).astype(fp8)
    w3 = np.ascontiguousarray(L3.T).astype(fp8)
    w4 = np.zeros((H, N4P), dtype=fp8)
    w4[:, :C_OUT] = L4.T.astype(fp8)

    shared = dict(
        w1h=w1h, w1l=w1l, w2=w2, w3=w3, w4=w4,
        ab1=abpack(a1, b1), ab2=abpack(a2, b2), ab3=abpack(a3, b3),
        ab4=abpack(a4, b4),
    )
    xt = inputs["x"].T  # [D_IN, B] view
    in_maps = []
    for c in range(NCORES):
        xs = np.ascontiguousarray(xt[:, c * BC:(c + 1) * BC], dtype=f32)
        xh = xs.astype(np.float16)
        r = xs - xh.astype(f32)
        xl = (r * f32(LO_SC)).astype(fp8)
        m = dict(shared)
        m["xh"] = xh
        m["xl"] = xl
        in_maps.append(m)
    return in_maps


def kernel(**inputs):
    from concourse.bass_utils import run_bass_kernel_spmd

    inputs = {k: np.asarray(v) for k, v in inputs.items()}
    if "nc" not in _CACHE:
        _CACHE["nc"] = _build_nc()
    nc = _CACHE["nc"]

    in_maps = _host_prep(inputs)
    res = run_bass_kernel_spmd(nc, in_maps, core_ids=list(range(NCORES)))

    out = np.empty((B, C_OUT), dtype=np.float32)
    for c in range(NCORES):
        out[c * BC:(c + 1) * BC, :] = res.results[c]["out_t"][:C_OUT, :].T
    return out


# revision 4
# speedup vs baseline: 1.2047x; 1.0203x over previous
"""TRN2 Bass kernel for the quantized 4-layer MLP (dense_mlp, 8 cores).

Strategy (v3):
  - Data-parallel over batch: each of the 8 cores gets 1024 of 8192 rows.
  - All weights quantized to integer LEVELS on host (bit-exact replica of
    the reference wquant: round(W/s) with RTNE), shipped as fp16 (layer-1
    hi) / fp8e4 (everything else). No on-device weight quantization.
  - All DRAM operands pre-arranged on host so each SBUF partition's data
    is contiguous in DRAM (fat DMA descriptors; the naive
    "(kt p) n -> p kt n" rearrange yields 128B lines at ~13GB/s).
  - Layer 1 x split: hi = fp16(x) [16 matmuls/tile], lo = fp8(r*2^9) with
    stationary levels*2^-9 (exact in fp8e4 subnormals) via DoubleRow
    [8 matmuls/tile]. hi/lo accumulate in separate PSUM banks (mixing
    perf modes in one accumulation group is broken on HW); the epilogue
    DVE scalar_tensor_tensor folds lo in: tmp = lo_psum*alpha + act(hi).
    Combined x precision ~2^-15; measured end-to-end rel err 1.04e-2
    (gate 2e-2). L1 runs in 2-nt groups (hi x4 then lo x4) to halve the
    fp16<->DoubleRow mode-switch penalty (~0.4us per switch).
  - Layers 2-4: fp8e4 DoubleRow matmuls over integer levels - bit-exact,
    2x tensor-engine throughput.
  - BN + QuantReLU epilogue fused: ACT per-feature affine, DVE round
    (+C/-C trick), DVE clip(15,0) with fp8 output cast.
  - Total matmuls/core: 1536 (L1) + 1024 (L2) + 1024 (L3) + 256 (L4)
    = 3840 @ ~216ns issue rate -> ~830us floor.
"""

import numpy as np
import ml_dtypes

B, D_IN, H, C_OUT = 8192, 2048, 4096, 1000
NCORES = 8
BC = B // NCORES            # 1024 batch rows per core
N4P = 1024                  # padded final output feature dim (1000 -> 1024)
C_ROUND = float(1.5 * 2 ** 23)
EPS = 1e-5
LO_SC = 512.0               # 2^9 residual scale for the fp8 lo pass
P = 128
KTX = D_IN // P             # 16
KT = H // P                 # 32
NT1 = H // P                # 32
NT4 = N4P // P              # 8

_CACHE = {}


def _build_nc():
    import concourse.bass as bass  # noqa: F401
    from concourse import bacc
    import concourse.mybir as mybir
    import concourse.tile as tile

    dt = mybir.dt
    AF = mybir.ActivationFunctionType
    ALU = mybir.AluOpType
    DR = mybir.MatmulPerfMode.DoubleRow

    nc = bacc.Bacc("TRN2", target_bir_lowering=False)

    # ---- DRAM I/O (all pre-arranged: partition-contiguous) ----
    xh_d = nc.dram_tensor("xh", [P, KTX * BC], dt.float16, kind="ExternalInput")
    xl_d = nc.dram_tensor("xl", [P, KTX * BC], dt.float8e4, kind="ExternalInput")
    w1h_d = nc.dram_tensor("w1h", [NT1, P, KTX * P], dt.float16, kind="ExternalInput")
    w1l_d = nc.dram_tensor("w1l", [NT1, P, KTX * P], dt.float8e4, kind="ExternalInput")
    w2_d = nc.dram_tensor("w2", [NT1, P, KT * P], dt.float8e4, kind="ExternalInput")
    w3_d = nc.dram_tensor("w3", [NT1, P, KT * P], dt.float8e4, kind="ExternalInput")
    w4_d = nc.dram_tensor("w4", [NT4, P, KT * P], dt.float8e4, kind="ExternalInput")
    ab1 = nc.dram_tensor("ab1", [P, NT1 * 2], dt.float32, kind="ExternalInput")
    ab2 = nc.dram_tensor("ab2", [P, NT1 * 2], dt.float32, kind="ExternalInput")
    ab3 = nc.dram_tensor("ab3", [P, NT1 * 2], dt.float32, kind="ExternalInput")
    ab4 = nc.dram_tensor("ab4", [P, NT4 * 2], dt.float32, kind="ExternalInput")
    out_t = nc.dram_tensor("out_t", [N4P, BC], dt.float32, kind="ExternalOutput")

    with tile.TileContext(nc) as tc:
        ppool_cm = tc.tile_pool(name="psum", bufs=4, space="PSUM")
        ppool = ppool_cm.__enter__()

        abt_cm = tc.tile_pool(name="abtp", bufs=1)
        abt_pool = abt_cm.__enter__()
        tmp_cm = tc.tile_pool(name="tmpp", bufs=4)
        tmp_pool = tmp_cm.__enter__()

        apool12_cm = tc.tile_pool(name="acts12", bufs=1)
        apool12 = apool12_cm.__enter__()
        A1 = apool12.tile([P, KT, BC], dt.float8e4, name="A1")

        def epilogue(psum, abt, nt, b0, a_out, lo_psum=None):
            tmp = tmp_pool.tile([P, 512], dt.float32, name="tmp", tag="tmp")
            if a_out is not None:
                nc.scalar.activation(
                    tmp[:], psum[:], AF.Identity,
                    bias=abt[:, 2 * nt + 1:2 * nt + 2], scale=abt[:, 2 * nt:2 * nt + 1])
                if lo_psum is not None:
                    nc.vector.scalar_tensor_tensor(
                        tmp[:], lo_psum[:], abt[:, 2 * nt:2 * nt + 1], tmp[:],
                        ALU.mult, ALU.add)
                nc.vector.tensor_scalar(tmp[:], tmp[:], C_ROUND, C_ROUND,
                                        ALU.add, ALU.subtract)
                nc.vector.tensor_scalar(a_out[:, nt, b0:b0 + 512], tmp[:],
                                        15.0, 0.0, ALU.min, ALU.max)
            else:
                ost = tmp_pool.tile([P, 512], dt.float32, name="ost", tag="ost")
                nc.scalar.activation(
                    ost[:], psum[:], AF.Identity,
                    bias=abt[:, 2 * nt + 1:2 * nt + 2], scale=abt[:, 2 * nt:2 * nt + 1])
                n0 = nt * P
                nc.gpsimd.dma_start(out_t[n0:n0 + P, b0:b0 + 512], ost[:])

        # ---- layer 1: fp16 hi + fp8 DR lo, separate psums ----
        xt_pool_cm = tc.tile_pool(name="xtp", bufs=1)
        xt_pool = xt_pool_cm.__enter__()
        xh_t = xt_pool.tile([P, KTX, BC], dt.float16, name="xh_t")
        xl_t = xt_pool.tile([P, KTX, BC], dt.float8e4, name="xl_t")

        w1_cm = tc.tile_pool(name="w1p", bufs=4)
        w1_pool = w1_cm.__enter__()

        abt1 = abt_pool.tile([P, NT1 * 2], dt.float32, name="abt1")

        def w1_fetch(nt):
            w1h_t = w1_pool.tile([P, KTX, P], dt.float16, name="w1h_t", tag="wh")
            w1l_t = w1_pool.tile([P, KTX, P], dt.float8e4, name="w1l_t", tag="wl")
            nc.sync.dma_start(
                w1h_t[:], w1h_d[nt].rearrange("p (kt n) -> p kt n", n=P))
            nc.sync.dma_start(
                w1l_t[:], w1l_d[nt].rearrange("p (kt n) -> p kt n", n=P))
            return w1h_t, w1l_t

        # DMA order: first weight tiles, then x (chunked, split across two
        # queues), so the first matmul can start ~5us in.
        w1_tiles = {0: w1_fetch(0), 1: w1_fetch(1)}
        for c0 in range(0, KTX, 4):
            q = nc.sync if (c0 // 4) % 2 == 0 else nc.gpsimd
            q.dma_start(xh_t[:, c0:c0 + 4, :],
                        xh_d[:, c0 * BC:(c0 + 4) * BC].rearrange(
                            "p (kt b) -> p kt b", b=BC))
            q2 = nc.gpsimd if (c0 // 4) % 2 == 0 else nc.sync
            q2.dma_start(xl_t[:, c0:c0 + 4, :],
                         xl_d[:, c0 * BC:(c0 + 4) * BC].rearrange(
                             "p (kt b) -> p kt b", b=BC))
        nc.gpsimd.dma_start(abt1[:], ab1[:])

        # process nt in pairs: hi x4 tasks, then lo x4 tasks (fewer
        # fp16<->DR transitions); psum tags ph/pl ring-4 = all 8 banks.
        for ntp in range(0, NT1, 2):
            for nt in (ntp + 2, ntp + 3):
                if nt < NT1:
                    w1_tiles[nt] = w1_fetch(nt)
            tasks = [(nt, b0) for nt in (ntp, ntp + 1) for b0 in (0, 512)]
            ps_h = {}
            ps_l = {}
            for nt, b0 in tasks:
                ph = ppool.tile([P, 512], dt.float32, name="ps_h", tag="ph")
                for kt in range(KTX):
                    nc.tensor.matmul(
                        ph[:], w1_tiles[nt][0][:, kt, :],
                        xh_t[:, kt, b0:b0 + 512],
                        start=(kt == 0), stop=(kt == KTX - 1))
                ps_h[(nt, b0)] = ph
            for nt, b0 in tasks:
                pl = ppool.tile([P, 512], dt.float32, name="ps_l", tag="pl")
                for kp in range(KTX // 2):
                    nc.tensor.matmul(
                        pl[:], w1_tiles[nt][1][:, 2 * kp:2 * kp + 2, :],
                        xl_t[:, 2 * kp:2 * kp + 2, b0:b0 + 512],
                        start=(kp == 0), stop=(kp == KTX // 2 - 1),
                        perf_mode=DR)
                ps_l[(nt, b0)] = pl
            for nt, b0 in tasks:
                epilogue(ps_h[(nt, b0)], abt1, nt, b0, A1,
                         lo_psum=ps_l[(nt, b0)])
            for nt in (ntp, ntp + 1):
                del w1_tiles[nt]

        w1_cm.__exit__(None, None, None)
        xt_pool_cm.__exit__(None, None, None)

        # ---- layers 2-4: fp8 DR with preloaded level weights ----
        apool23_cm = tc.tile_pool(name="acts23", bufs=1)
        apool23 = apool23_cm.__enter__()
        A2 = apool23.tile([P, KT, BC], dt.float8e4, name="A2")
        A3 = apool23.tile([P, KT, BC], dt.float8e4, name="A3")

        wf_cm = tc.tile_pool(name="wfp", bufs=3)
        wf_pool = wf_cm.__enter__()

        def layer_dr(wt, abt, NT, a_in, a_out, idx):
            for nt in range(NT):
                qt = wf_pool.tile([P, KT, P], dt.float8e4, name=f"qt{idx}",
                                  tag="qt")
                nc.sync.dma_start(
                    qt[:], wt[nt].rearrange("p (kt n) -> p kt n", n=P))
                for b0 in (0, 512):
                    psum = ppool.tile([P, 512], dt.float32, name="ps", tag="ph")
                    for kp in range(KT // 2):
                        nc.tensor.matmul(
                            psum[:], qt[:, 2 * kp:2 * kp + 2, :],
                            a_in[:, 2 * kp:2 * kp + 2, b0:b0 + 512],
                            start=(kp == 0), stop=(kp == KT // 2 - 1),
                            perf_mode=DR)
                    epilogue(psum, abt, nt, b0, a_out)

        abt2 = abt_pool.tile([P, NT1 * 2], dt.float32, name="abt2")
        nc.gpsimd.dma_start(abt2[:], ab2[:])
        layer_dr(w2_d, abt2, NT1, A1, A2, 2)
        abt3 = abt_pool.tile([P, NT1 * 2], dt.float32, name="abt3")
        nc.gpsimd.dma_start(abt3[:], ab3[:])
        layer_dr(w3_d, abt3, NT1, A2, A3, 3)
        abt4 = abt_pool.tile([P, NT4 * 2], dt.float32, name="abt4")
        nc.gpsimd.dma_start(abt4[:], ab4[:])
        layer_dr(w4_d, abt4, NT4, A3, None, 4)

        wf_cm.__exit__(None, None, None)
        apool23_cm.__exit__(None, None, None)
        apool12_cm.__exit__(None, None, None)
        tmp_cm.__exit__(None, None, None)
        abt_cm.__exit__(None, None, None)
        ppool_cm.__exit__(None, None, None)

    nc.compile()
    return nc


def _arr_w(Wt, NT):
    """[K, N] -> [NT, P, KT*P] with partition-contiguous per-nt blocks."""
    K = Wt.shape[0]
    kt = K // P
    # [K, N] -> (kt, P, NT, P) -> (NT, P_part, kt, P_n)
    a = Wt.reshape(kt, P, NT, P).transpose(2, 1, 0, 3).reshape(NT, P, kt * P)
    return np.ascontiguousarray(a)


def _host_prep(inputs):
    f32 = np.float32
    fp8 = ml_dtypes.float8_e4m3

    def levels(W):
        s = f32(np.max(np.abs(W))) / f32(3.0)
        return np.clip(np.round(W / s), -3.0, 3.0).astype(f32), s

    L1, sw1 = levels(inputs["W1"])
    L2, sw2 = levels(inputs["W2"])
    L3, sw3 = levels(inputs["W3"])
    L4, sw4 = levels(inputs["W4"])
    s_a = [f32(inputs[k][0]) for k in ("s1", "s2", "s3")]

    def fold(l, s_w, s_prev):
        g = inputs[f"g{l}"].astype(np.float64)
        be = inputs[f"be{l}"].astype(np.float64)
        m = inputs[f"m{l}"].astype(np.float64)
        v = inputs[f"v{l}"].astype(np.float64)
        b = inputs[f"b{l}"].astype(np.float64)
        inv = 1.0 / np.sqrt(v + EPS)
        sl = float(s_a[l - 1])
        alpha = (float(s_prev) * float(s_w) * g * inv) / sl
        beta = ((b - m) * inv * g + be) / sl
        return alpha.astype(f32), beta.astype(f32)

    a1, b1 = fold(1, sw1, 1.0)
    a2, b2 = fold(2, sw2, s_a[0])
    a3, b3 = fold(3, sw3, s_a[1])
    a4 = np.full(N4P, float(s_a[2]) * float(sw4), dtype=f32)
    b4 = np.zeros(N4P, dtype=f32)
    b4[:C_OUT] = inputs["b4"]

    def abpack(a, b, NT):
        # [N] alpha, [N] beta -> [P, NT*2] with (alpha, beta) interleaved
        ab = np.stack([a, b], axis=1).reshape(NT, P, 2)
        return np.ascontiguousarray(ab.transpose(1, 0, 2).reshape(P, NT * 2))

    w1h = _arr_w(L1.T, NT1).astype(np.float16)
    w1l = _arr_w(L1.T * f32(1.0 / LO_SC), NT1).astype(fp8)
    w2 = _arr_w(L2.T, NT1).astype(fp8)
    w3 = _arr_w(L3.T, NT1).astype(fp8)
    L4p = np.zeros((N4P, H), dtype=f32)
    L4p[:C_OUT] = L4
    w4 = _arr_w(L4p.T, NT4).astype(fp8)

    shared = dict(
        w1h=w1h, w1l=w1l, w2=w2, w3=w3, w4=w4,
        ab1=abpack(a1, b1, NT1), ab2=abpack(a2, b2, NT1),
        ab3=abpack(a3, b3, NT1), ab4=abpack(a4, b4, NT4),
    )
    xt = inputs["x"].T  # [D_IN, B] view
    in_maps = []
    for c in range(NCORES):
        xs = np.ascontiguousarray(xt[:, c * BC:(c + 1) * BC], dtype=f32)
        xh = xs.astype(np.float16)
        r = xs - xh.astype(f32)
        xl = (r * f32(LO_SC)).astype(fp8)
        # [D_IN, BC] -> [P, KTX*BC] partition-contiguous
        m = dict(shared)
        m["xh"] = np.ascontiguousarray(
            xh.reshape(KTX, P, BC).transpose(1, 0, 2).reshape(P, KTX * BC))
        m["xl"] = np.ascontiguousarray(
            xl.reshape(KTX, P, BC).transpose(1, 0, 2).reshape(P, KTX * BC))
        in_maps.append(m)
    return in_maps


def kernel(**inputs):
    from concourse.bass_utils import run_bass_kernel_spmd

    inputs = {k: np.asarray(v) for k, v in inputs.items()}
    if "nc" not in _CACHE:
        _CACHE["nc"] = _build_nc()
    nc = _CACHE["nc"]

    in_maps = _host_prep(inputs)
    res = run_bass_kernel_spmd(nc, in_maps, core_ids=list(range(NCORES)))

    out = np.empty((B, C_OUT), dtype=np.float32)
    for c in range(NCORES):
        out[c * BC:(c + 1) * BC, :] = res.results[c]["out_t"][:C_OUT, :].T
    return out


# revision 5
# speedup vs baseline: 1.2109x; 1.0052x over previous
"""TRN2 Bass kernel for the quantized 4-layer MLP (dense_mlp, 8 cores).

Strategy (v3):
  - Data-parallel over batch: each of the 8 cores gets 1024 of 8192 rows.
  - All weights quantized to integer LEVELS on host (bit-exact replica of
    the reference wquant: round(W/s) with RTNE), shipped as fp16 (layer-1
    hi) / fp8e4 (everything else). No on-device weight quantization.
  - All DRAM operands pre-arranged on host so each SBUF partition's data
    is contiguous in DRAM (fat DMA descriptors; the naive
    "(kt p) n -> p kt n" rearrange yields 128B lines at ~13GB/s).
  - Layer 1 x split: hi = fp16(x) [16 matmuls/tile], lo = fp8(r*2^9) with
    stationary levels*2^-9 (exact in fp8e4 subnormals) via DoubleRow
    [8 matmuls/tile]. hi/lo accumulate in separate PSUM banks (mixing
    perf modes in one accumulation group is broken on HW); the epilogue
    DVE scalar_tensor_tensor folds lo in: tmp = lo_psum*alpha + act(hi).
    Combined x precision ~2^-15; measured end-to-end rel err 1.04e-2
    (gate 2e-2). L1 runs in 2-nt groups (hi x4 then lo x4) to halve the
    fp16<->DoubleRow mode-switch penalty (~0.4us per switch).
  - Layers 2-4: fp8e4 DoubleRow matmuls over integer levels - bit-exact,
    2x tensor-engine throughput.
  - BN + QuantReLU epilogue fused: ACT per-feature affine, DVE round
    (+C/-C trick), DVE clip(15,0) with fp8 output cast.
  - Total matmuls/core: 1536 (L1) + 1024 (L2) + 1024 (L3) + 256 (L4)
    = 3840 @ ~216ns issue rate -> ~830us floor.
"""

import numpy as np
import ml_dtypes

B, D_IN, H, C_OUT = 8192, 2048, 4096, 1000
NCORES = 8
BC = B // NCORES            # 1024 batch rows per core
N4P = 1024                  # padded final output feature dim (1000 -> 1024)
C_ROUND = float(1.5 * 2 ** 23)
EPS = 1e-5
LO_SC = 512.0               # 2^9 residual scale for the fp8 lo pass
P = 128
KTX = D_IN // P             # 16
KT = H // P                 # 32
NT1 = H // P                # 32
NT4 = N4P // P              # 8

_CACHE = {}


def _build_nc():
    import concourse.bass as bass  # noqa: F401
    from concourse import bacc
    import concourse.mybir as mybir
    import concourse.tile as tile

    dt = mybir.dt
    AF = mybir.ActivationFunctionType
    ALU = mybir.AluOpType
    DR = mybir.MatmulPerfMode.DoubleRow

    nc = bacc.Bacc("TRN2", target_bir_lowering=False)

    # ---- DRAM I/O (all pre-arranged: partition-contiguous) ----
    xh_d = nc.dram_tensor("xh", [P, KTX * BC], dt.float16, kind="ExternalInput")
    xl_d = nc.dram_tensor("xl", [P, KTX * BC], dt.float8e4, kind="ExternalInput")
    w1h_d = nc.dram_tensor("w1h", [NT1, P, KTX * P], dt.float16, kind="ExternalInput")
    w1l_d = nc.dram_tensor("w1l", [NT1, P, KTX * P], dt.float8e4, kind="ExternalInput")
    w2_d = nc.dram_tensor("w2", [NT1, P, KT * P], dt.float8e4, kind="ExternalInput")
    w3_d = nc.dram_tensor("w3", [NT1, P, KT * P], dt.float8e4, kind="ExternalInput")
    w4_d = nc.dram_tensor("w4", [NT4, P, KT * P], dt.float8e4, kind="ExternalInput")
    ab1 = nc.dram_tensor("ab1", [P, NT1 * 2], dt.float32, kind="ExternalInput")
    ab2 = nc.dram_tensor("ab2", [P, NT1 * 2], dt.float32, kind="ExternalInput")
    ab3 = nc.dram_tensor("ab3", [P, NT1 * 2], dt.float32, kind="ExternalInput")
    ab4 = nc.dram_tensor("ab4", [P, NT4 * 2], dt.float32, kind="ExternalInput")
    out_t = nc.dram_tensor("out_t", [N4P, BC], dt.float32, kind="ExternalOutput")

    with tile.TileContext(nc) as tc:
        ppool_cm = tc.tile_pool(name="psum", bufs=4, space="PSUM")
        ppool = ppool_cm.__enter__()

        abt_cm = tc.tile_pool(name="abtp", bufs=1)
        abt_pool = abt_cm.__enter__()
        tmp_cm = tc.tile_pool(name="tmpp", bufs=4)
        tmp_pool = tmp_cm.__enter__()

        apool12_cm = tc.tile_pool(name="acts12", bufs=1)
        apool12 = apool12_cm.__enter__()
        A1 = apool12.tile([P, KT, BC], dt.float8e4, name="A1")

        def epilogue(psum, abt, nt, b0, a_out, lo_psum=None):
            tmp = tmp_pool.tile([P, 512], dt.float32, name="tmp", tag="tmp")
            if a_out is not None:
                nc.scalar.activation(
                    tmp[:], psum[:], AF.Identity,
                    bias=abt[:, 2 * nt + 1:2 * nt + 2], scale=abt[:, 2 * nt:2 * nt + 1])
                if lo_psum is not None:
                    nc.vector.scalar_tensor_tensor(
                        tmp[:], lo_psum[:], abt[:, 2 * nt:2 * nt + 1], tmp[:],
                        ALU.mult, ALU.add)
                nc.vector.tensor_scalar(tmp[:], tmp[:], C_ROUND, C_ROUND,
                                        ALU.add, ALU.subtract)
                nc.vector.tensor_scalar(a_out[:, nt, b0:b0 + 512], tmp[:],
                                        15.0, 0.0, ALU.min, ALU.max)
            else:
                ost = tmp_pool.tile([P, 512], dt.float32, name="ost", tag="ost")
                nc.scalar.activation(
                    ost[:], psum[:], AF.Identity,
                    bias=abt[:, 2 * nt + 1:2 * nt + 2], scale=abt[:, 2 * nt:2 * nt + 1])
                n0 = nt * P
                nc.gpsimd.dma_start(out_t[n0:n0 + P, b0:b0 + 512], ost[:])

        # ---- layer 1: fp16 hi + fp8 DR lo, separate psums ----
        xt_pool_cm = tc.tile_pool(name="xtp", bufs=1)
        xt_pool = xt_pool_cm.__enter__()
        xh_t = xt_pool.tile([P, KTX, BC], dt.float16, name="xh_t")
        xl_t = xt_pool.tile([P, KTX, BC], dt.float8e4, name="xl_t")

        w1_cm = tc.tile_pool(name="w1p", bufs=4)
        w1_pool = w1_cm.__enter__()
        w1l_cm = tc.tile_pool(name="w1lp", bufs=9)
        w1l_pool = w1l_cm.__enter__()
        l1tmp_cm = tc.tile_pool(name="l1tmp", bufs=16)
        l1tmp_pool = l1tmp_cm.__enter__()

        abt1 = abt_pool.tile([P, NT1 * 2], dt.float32, name="abt1")

        def w1_fetch(nt):
            w1h_t = w1_pool.tile([P, KTX, P], dt.float16, name="w1h_t", tag="wh")
            w1l_t = w1l_pool.tile([P, KTX, P], dt.float8e4, name="w1l_t", tag="wl")
            nc.sync.dma_start(
                w1h_t[:], w1h_d[nt].rearrange("p (kt n) -> p kt n", n=P))
            nc.sync.dma_start(
                w1l_t[:], w1l_d[nt].rearrange("p (kt n) -> p kt n", n=P))
            return w1h_t, w1l_t

        # DMA order: first nt0 hi weights, then all xh (hi pass input,
        # alternating queues), then xl (needed only ~55us in), so the
        # first matmul starts ~5us in and the hi pass streams.
        w1_tiles = {0: w1_fetch(0)}
        for c0 in range(0, KTX, 2):
            q = nc.sync if (c0 // 2) % 2 == 0 else nc.gpsimd
            q.dma_start(xh_t[:, c0:c0 + 2, :],
                        xh_d[:, c0 * BC:(c0 + 2) * BC].rearrange(
                            "p (kt b) -> p kt b", b=BC))
        for c0 in range(0, KTX, 4):
            q = nc.gpsimd if (c0 // 4) % 2 == 0 else nc.sync
            q.dma_start(xl_t[:, c0:c0 + 4, :],
                        xl_d[:, c0 * BC:(c0 + 4) * BC].rearrange(
                            "p (kt b) -> p kt b", b=BC))
        nc.gpsimd.dma_start(abt1[:], ab1[:])

        # process nt in groups of 8: hi passes (ACT spills psum to SBUF
        # right away), then lo passes + combine epilogue. 2 fp16<->DR
        # mode switches per group instead of per nt.
        G1 = 8
        for ntp in range(0, NT1, G1):
            grp = range(ntp, ntp + G1)
            tasks = [(nt, b0) for nt in grp for b0 in (0, 512)]
            tmps = {}
            for nt, b0 in tasks:
                if b0 == 0 and nt + 1 not in w1_tiles and nt + 1 < NT1:
                    w1_tiles[nt + 1] = w1_fetch(nt + 1)
                ph = ppool.tile([P, 512], dt.float32, name="ps_h", tag="ph")
                for kt in range(KTX):
                    nc.tensor.matmul(
                        ph[:], w1_tiles[nt][0][:, kt, :],
                        xh_t[:, kt, b0:b0 + 512],
                        start=(kt == 0), stop=(kt == KTX - 1))
                tmp = l1tmp_pool.tile([P, 512], dt.float32, name="l1t", tag="l1t")
                nc.scalar.activation(
                    tmp[:], ph[:], AF.Identity,
                    bias=abt1[:, 2 * nt + 1:2 * nt + 2],
                    scale=abt1[:, 2 * nt:2 * nt + 1])
                tmps[(nt, b0)] = tmp
            for nt, b0 in tasks:
                pl = ppool.tile([P, 512], dt.float32, name="ps_l", tag="pl")
                for kp in range(KTX // 2):
                    nc.tensor.matmul(
                        pl[:], w1_tiles[nt][1][:, 2 * kp:2 * kp + 2, :],
                        xl_t[:, 2 * kp:2 * kp + 2, b0:b0 + 512],
                        start=(kp == 0), stop=(kp == KTX // 2 - 1),
                        perf_mode=DR)
                tmp = tmps[(nt, b0)]
                nc.vector.scalar_tensor_tensor(
                    tmp[:], pl[:], abt1[:, 2 * nt:2 * nt + 1], tmp[:],
                    ALU.mult, ALU.add)
                nc.vector.tensor_scalar(tmp[:], tmp[:], C_ROUND, C_ROUND,
                                        ALU.add, ALU.subtract)
                nc.vector.tensor_scalar(A1[:, nt, b0:b0 + 512], tmp[:],
                                        15.0, 0.0, ALU.min, ALU.max)
            for nt in grp:
                del w1_tiles[nt]

        l1tmp_cm.__exit__(None, None, None)
        w1l_cm.__exit__(None, None, None)
        w1_cm.__exit__(None, None, None)
        xt_pool_cm.__exit__(None, None, None)

        # ---- layers 2-4: fp8 DR with preloaded level weights ----
        apool23_cm = tc.tile_pool(name="acts23", bufs=1)
        apool23 = apool23_cm.__enter__()
        A2 = apool23.tile([P, KT, BC], dt.float8e4, name="A2")
        A3 = apool23.tile([P, KT, BC], dt.float8e4, name="A3")

        wf_cm = tc.tile_pool(name="wfp", bufs=3)
        wf_pool = wf_cm.__enter__()

        def layer_dr(wt, abt, NT, a_in, a_out, idx):
            for nt in range(NT):
                qt = wf_pool.tile([P, KT, P], dt.float8e4, name=f"qt{idx}",
                                  tag="qt")
                nc.sync.dma_start(
                    qt[:], wt[nt].rearrange("p (kt n) -> p kt n", n=P))
                for b0 in (0, 512):
                    psum = ppool.tile([P, 512], dt.float32, name="ps", tag="ph")
                    for kp in range(KT // 2):
                        nc.tensor.matmul(
                            psum[:], qt[:, 2 * kp:2 * kp + 2, :],
                            a_in[:, 2 * kp:2 * kp + 2, b0:b0 + 512],
                            start=(kp == 0), stop=(kp == KT // 2 - 1),
                            perf_mode=DR)
                    epilogue(psum, abt, nt, b0, a_out)

        abt2 = abt_pool.tile([P, NT1 * 2], dt.float32, name="abt2")
        nc.gpsimd.dma_start(abt2[:], ab2[:])
        layer_dr(w2_d, abt2, NT1, A1, A2, 2)
        abt3 = abt_pool.tile([P, NT1 * 2], dt.float32, name="abt3")
        nc.gpsimd.dma_start(abt3[:], ab3[:])
        layer_dr(w3_d, abt3, NT1, A2, A3, 3)
        abt4 = abt_pool.tile([P, NT4 * 2], dt.float32, name="abt4")
        nc.gpsimd.dma_start(abt4[:], ab4[:])
        layer_dr(w4_d, abt4, NT4, A3, None, 4)

        wf_cm.__exit__(None, None, None)
        apool23_cm.__exit__(None, None, None)
        apool12_cm.__exit__(None, None, None)
        tmp_cm.__exit__(None, None, None)
        abt_cm.__exit__(None, None, None)
        ppool_cm.__exit__(None, None, None)

    nc.compile()
    return nc


def _arr_w(Wt, NT):
    """[K, N] -> [NT, P, KT*P] with partition-contiguous per-nt blocks."""
    K = Wt.shape[0]
    kt = K // P
    # [K, N] -> (kt, P, NT, P) -> (NT, P_part, kt, P_n)
    a = Wt.reshape(kt, P, NT, P).transpose(2, 1, 0, 3).reshape(NT, P, kt * P)
    return np.ascontiguousarray(a)


def _host_prep(inputs):
    f32 = np.float32
    fp8 = ml_dtypes.float8_e4m3

    def levels(W):
        s = f32(np.max(np.abs(W))) / f32(3.0)
        return np.clip(np.round(W / s), -3.0, 3.0).astype(f32), s

    L1, sw1 = levels(inputs["W1"])
    L2, sw2 = levels(inputs["W2"])
    L3, sw3 = levels(inputs["W3"])
    L4, sw4 = levels(inputs["W4"])
    s_a = [f32(inputs[k][0]) for k in ("s1", "s2", "s3")]

    def fold(l, s_w, s_prev):
        g = inputs[f"g{l}"].astype(np.float64)
        be = inputs[f"be{l}"].astype(np.float64)
        m = inputs[f"m{l}"].astype(np.float64)
        v = inputs[f"v{l}"].astype(np.float64)
        b = inputs[f"b{l}"].astype(np.float64)
        inv = 1.0 / np.sqrt(v + EPS)
        sl = float(s_a[l - 1])
        alpha = (float(s_prev) * float(s_w) * g * inv) / sl
        beta = ((b - m) * inv * g + be) / sl
        return alpha.astype(f32), beta.astype(f32)

    a1, b1 = fold(1, sw1, 1.0)
    a2, b2 = fold(2, sw2, s_a[0])
    a3, b3 = fold(3, sw3, s_a[1])
    a4 = np.full(N4P, float(s_a[2]) * float(sw4), dtype=f32)
    b4 = np.zeros(N4P, dtype=f32)
    b4[:C_OUT] = inputs["b4"]

    def abpack(a, b, NT):
        # [N] alpha, [N] beta -> [P, NT*2] with (alpha, beta) interleaved
        ab = np.stack([a, b], axis=1).reshape(NT, P, 2)
        return np.ascontiguousarray(ab.transpose(1, 0, 2).reshape(P, NT * 2))

    w1h = _arr_w(L1.T, NT1).astype(np.float16)
    w1l = _arr_w(L1.T * f32(1.0 / LO_SC), NT1).astype(fp8)
    w2 = _arr_w(L2.T, NT1).astype(fp8)
    w3 = _arr_w(L3.T, NT1).astype(fp8)
    L4p = np.zeros((N4P, H), dtype=f32)
    L4p[:C_OUT] = L4
    w4 = _arr_w(L4p.T, NT4).astype(fp8)

    shared = dict(
        w1h=w1h, w1l=w1l, w2=w2, w3=w3, w4=w4,
        ab1=abpack(a1, b1, NT1), ab2=abpack(a2, b2, NT1),
        ab3=abpack(a3, b3, NT1), ab4=abpack(a4, b4, NT4),
    )
    xt = inputs["x"].T  # [D_IN, B] view
    in_maps = []
    for c in range(NCORES):
        xs = np.ascontiguousarray(xt[:, c * BC:(c + 1) * BC], dtype=f32)
        xh = xs.astype(np.float16)
        r = xs - xh.astype(f32)
        xl = (r * f32(LO_SC)).astype(fp8)
        # [D_IN, BC] -> [P, KTX*BC] partition-contiguous
        m = dict(shared)
        m["xh"] = np.ascontiguousarray(
            xh.reshape(KTX, P, BC).transpose(1, 0, 2).reshape(P, KTX * BC))
        m["xl"] = np.ascontiguousarray(
            xl.reshape(KTX, P, BC).transpose(1, 0, 2).reshape(P, KTX * BC))
        in_maps.append(m)
    return in_maps


def kernel(**inputs):
    from concourse.bass_utils import run_bass_kernel_spmd

    inputs = {k: np.asarray(v) for k, v in inputs.items()}
    if "nc" not in _CACHE:
        _CACHE["nc"] = _build_nc()
    nc = _CACHE["nc"]

    in_maps = _host_prep(inputs)
    res = run_bass_kernel_spmd(nc, in_maps, core_ids=list(range(NCORES)))

    out = np.empty((B, C_OUT), dtype=np.float32)
    for c in range(NCORES):
        out[c * BC:(c + 1) * BC, :] = res.results[c]["out_t"][:C_OUT, :].T
    return out
